# revision 1
# baseline (speedup 1.0000x reference)
"""Trainium2 Bass kernel for CarlosSelfAttention (B=2, T=2048, C=1024, H=16).

Sharding: tensor-parallel over heads. 8 cores x 2 heads each.
Each core computes q/k/v projections for its 2 heads, RoPE, causal
attention, and a partial out-projection against its 128 columns of Wo.
The host sums the 8 partial outputs (the TP all-reduce) and adds the
output bias plus the (v-bias @ Wo.T) correction term.

All on-chip layouts are "transposed" ([dim, token]) so every matmul
contraction lands on the partition axis:
  xT   [1024, 4096]   (input, replicated)
  qT/kT[128, 4096]    rows = [h0-even dims, h0-odd, h1-even, h1-odd]
  vT   2 x [64, 4096] rows = plain head dims
  S^T  [k-tile 128, q-chunk 512] via PE, exp'd on ScalarE from PSUM
  P@V  col-packed (h0 -> psum rows 0:63, h1 -> 64:127), sums via
       ones-matmul, normalization by reciprocal broadcast.
  out  y_part [4096, 1024] = OT.T @ WoT via PE, DMA'd from PSUM.
"""

import os
import numpy as np

import concourse.bass as bass
import concourse.tile as tile
from concourse import bacc, mybir
from concourse.bass_utils import run_bass_kernel_spmd

F32 = mybir.dt.float32
F32R = mybir.dt.float32r
AF = mybir.ActivationFunctionType

B, T, C, H, HD = 2, 2048, 1024, 16, 64
NCORES = 8
TB = B * T          # 4096
QCH = 512           # q-chunk (moving dim)
NQC = T // QCH      # 4 q-chunks per batch
NKT = T // 128      # 16 k-tiles per batch
NTC = TB // QCH     # 8 t-chunks for the projections
NCT = C // 128      # 8 contraction tiles

_PROG_CACHE: dict = {}


def _emit(tc, mode, dram):
    nc = tc.nc
    from contextlib import ExitStack

    xT, wT, bqk, cosT, sinS, woT, y = (
        dram["xT"], dram["wT"], dram["bqk"], dram["cosT"], dram["sinS"],
        dram["woT"], dram["y"])
    maskT = dram.get("maskT")

    with ExitStack() as ctx:
        constp = ctx.enter_context(tc.tile_pool(name="const", bufs=1))
        pers = ctx.enter_context(tc.tile_pool(name="pers", bufs=1))

        # ---- constants ----
        wsb = constp.tile([128, NCT, 384], F32)
        nc.sync.dma_start(wsb[:].bitcast(F32R),
                          wT[:].rearrange("(a p) m -> p a m", p=128).bitcast(F32R))
        cos_sb = constp.tile([128, T], F32)
        nc.sync.dma_start(cos_sb[:], cosT[:])
        sin_sb = constp.tile([128, T], F32)
        nc.sync.dma_start(sin_sb[:], sinS[:])
        bqk_sb = constp.tile([128, 2], F32)
        nc.sync.dma_start(bqk_sb[:], bqk[:])
        wo_sb = constp.tile([128, C], F32)
        nc.sync.dma_start(wo_sb[:].bitcast(F32R), woT[:].bitcast(F32R))
        ones16 = constp.tile([128, NKT], F32)
        nc.vector.memset(ones16[:], 1.0)
        id64 = constp.tile([64, 64], F32)
        nc.vector.memset(id64[:], 1.0)
        nc.gpsimd.affine_select(
            out=id64[:], in_=id64[:], compare_op=mybir.AluOpType.is_equal,
            fill=0.0, base=0, channel_multiplier=1, pattern=[[-1, 64]])

        # ---- persistent activations ----
        qT = pers.tile([128, TB], F32)
        kT = pers.tile([128, TB], F32)
        vTf = pers.tile([128, TB], F32)
        vT1 = pers.tile([64, TB], F32)
        Vsb = [[pers.tile([128, NKT * (HD + 1)], F32, name=f"Vsb{b}{h}")
                for h in range(2)] for b in range(B)]
        OT = [pers.tile([128, T], F32, name=f"OTb{b}") for b in range(B)]

        def qkv_pair(xp, psqkv, tca, tcb, defer=None):
            """Emit qkv projection for two t-chunks with shared stationaries.
            If defer is a list, append per-(g,ct) units instead of emitting."""
            tsa = slice(tca * QCH, (tca + 1) * QCH)
            tsb = slice(tcb * QCH, (tcb + 1) * QCH)
            xa, xb = [], []
            def load(ct, tci, ts, lst):
                xt = xp.tile([128, QCH], F32, tag="x", name=f"xt{tci}_{ct}")
                nc.sync.dma_start(
                    xt[:].bitcast(F32R),
                    xT[ct * 128:(ct + 1) * 128, ts].bitcast(F32R))
                lst.append(xt)
            def emit_g(g):
                psa = psqkv.tile([128, QCH], F32, tag="ps",
                                 name=f"psq{tca}_{g}")
                psb = psqkv.tile([128, QCH], F32, tag="ps",
                                 name=f"psq{tcb}_{g}")
                def emit_ct(g, ct, psa=psa, psb=psb):
                    w = wsb[:, ct, g * 128:(g + 1) * 128].bitcast(F32R)
                    nc.tensor.matmul(psa[:], w, xa[ct][:].bitcast(F32R),
                                     start=(ct == 0), stop=(ct == NCT - 1))
                    nc.tensor.matmul(psb[:], w, xb[ct][:].bitcast(F32R),
                                     start=(ct == 0), stop=(ct == NCT - 1))
                def evict(g, ps, ts):
                    if g == 0:
                        nc.scalar.activation(qT[:, ts].bitcast(F32R), ps[:],
                                             AF.Identity, bias=bqk_sb[:, 0:1])
                    elif g == 1:
                        nc.scalar.activation(kT[:, ts].bitcast(F32R), ps[:],
                                             AF.Identity, bias=bqk_sb[:, 1:2])
                    else:
                        nc.scalar.activation(vTf[:, ts], ps[:], AF.Copy)
                        nc.sync.dma_start(vT1[:, ts], vTf[64:128, ts])
                for ct in range(NCT):
                    if defer is None:
                        emit_ct(g, ct)
                    else:
                        defer.append(lambda g=g, ct=ct: emit_ct(g, ct))
                if defer is None:
                    evict(g, psa, tsa)
                    evict(g, psb, tsb)
                else:
                    defer.append(lambda g=g, psa=psa, tsa=tsa: evict(g, psa, tsa))
                    defer.append(lambda g=g, psb=psb, tsb=tsb: evict(g, psb, tsb))
            for ct in range(NCT):
                load(ct, tca, tsa, xa)
                load(ct, tcb, tsb, xb)
            for g in range(3):
                emit_g(g)
            return None

        def rope_b(swpp, rtp, zt, b, nm):
            bs = slice(b * T, (b + 1) * T)
            swp = swpp.tile([128, T], F32, tag="swp", name=f"swp{nm}")
            for h in range(2):
                o = h * 64
                nc.sync.dma_start(swp[o:o + 32, :], zt[o + 32:o + 64, bs])
                nc.sync.dma_start(swp[o + 32:o + 64, :], zt[o:o + 32, bs])
            tmp = rtp.tile([128, T], F32, tag="rt", name=f"rt{nm}")
            nc.vector.tensor_mul(tmp[:], swp[:], sin_sb[:])
            nc.vector.tensor_mul(zt[:, bs].bitcast(F32R), zt[:, bs], cos_sb[:])
            nc.vector.tensor_add(zt[:, bs].bitcast(F32R), zt[:, bs], tmp[:])

        def vtrans_b(pstr, b):
            for h, vt in ((0, vTf), (1, vT1)):
                vov = Vsb[b][h][:].rearrange("p (t c) -> p t c", c=HD + 1)
                nc.vector.tensor_copy(vov[:, :, HD:HD + 1].bitcast(F32R),
                                      ones16[:])
                for tt in range(NKT):
                    pst = pstr.tile([128, 64], F32, tag="tr",
                                    name=f"pst{b}{h}{tt}")
                    nc.tensor.transpose(
                        pst[:],
                        vt[0:64, b * T + tt * 128: b * T + (tt + 1) * 128],
                        id64[:])
                    nc.vector.tensor_copy(
                        Vsb[b][h][:, tt * (HD + 1):tt * (HD + 1) + HD]
                        .bitcast(F32R),
                        pst[:])

        def attn_b(pools, b, fillers, qc_done=None):
            pss, pso, ptp, mbp, smol, bcp = pools
            PIPE = 3
            for qc in range(NQC):
                nk = 4 * (qc + 1) if mode == "causal" else NKT
                qs = slice(b * T + qc * QCH, b * T + (qc + 1) * QCH)
                psO0 = pso.tile([65, QCH], F32, tag="o0", name=f"psO0_{b}{qc}")
                psO1 = pso.tile([65, QCH], F32, tag="o1", name=f"psO1_{b}{qc}")
                pts = {}

                def emit_pv(j, nk=nk, psO0=psO0, psO1=psO1, pts=pts):
                    st, sp = (j == 0), (j == nk - 1)
                    pt = pts.pop(j)
                    nc.tensor.matmul(
                        psO0[:],
                        Vsb[b][0][:, j * (HD + 1):(j + 1) * (HD + 1)]
                        .bitcast(F32R),
                        pt[:, 0:QCH].bitcast(F32R), start=st, stop=sp)
                    nc.tensor.matmul(
                        psO1[:],
                        Vsb[b][1][:, j * (HD + 1):(j + 1) * (HD + 1)]
                        .bitcast(F32R),
                        pt[:, QCH:2 * QCH].bitcast(F32R), start=st, stop=sp)

                for kt in range(nk):
                    ks = slice(b * T + kt * 128, b * T + (kt + 1) * 128)
                    psS = pss.tile([128, 2 * QCH], F32, tag="s",
                                   name=f"psS{b}{qc}{kt}")
                    nc.tensor.matmul(psS[:, 0:QCH],
                                     kT[0:64, ks].bitcast(F32R),
                                     qT[0:64, qs].bitcast(F32R),
                                     start=True, stop=True)
                    nc.tensor.matmul(psS[:, QCH:2 * QCH],
                                     kT[64:128, ks].bitcast(F32R),
                                     qT[64:128, qs].bitcast(F32R),
                                     start=True, stop=True)
                    pt = ptp.tile([128, 2 * QCH], F32, tag="pt",
                                  name=f"pt{b}{qc}{kt}")
                    nc.scalar.activation(pt[:].bitcast(F32R), psS[:], AF.Exp)
                    if mode == "causal" and kt >= 4 * qc:
                        base = qc * QCH - kt * 128
                        ptv = pt[:].rearrange("p (h q) -> p h q", q=QCH)
                        nc.gpsimd.affine_select(
                            out=ptv.bitcast(F32R), in_=ptv.bitcast(F32R),
                            compare_op=mybir.AluOpType.is_ge,
                            fill=0.0, base=base, channel_multiplier=-1,
                            pattern=[[0, 2], [1, QCH]])
                    elif mode == "bias":
                        mt = mbp.tile([128, QCH], F32, tag="mb",
                                      name=f"mt{b}{qc}{kt}")
                        nc.sync.dma_start(
                            mt[:], maskT[kt * 128:(kt + 1) * 128,
                                         qc * QCH:(qc + 1) * QCH])
                        nc.vector.tensor_mul(pt[:, 0:QCH].bitcast(F32R),
                                             pt[:, 0:QCH], mt[:])
                        nc.vector.tensor_mul(pt[:, QCH:2 * QCH].bitcast(F32R),
                                             pt[:, QCH:2 * QCH], mt[:])
                    pts[kt] = pt
                    if fillers:
                        fillers.popleft()()
                    if kt >= PIPE:
                        emit_pv(kt - PIPE)
                for j in range(max(0, nk - PIPE), nk):
                    emit_pv(j)

                # normalize + evict; sum(exp) in row 64 of psO*
                oqs = slice(qc * QCH, (qc + 1) * QCH)
                for h, psO in ((0, psO0), (1, psO1)):
                    nm = f"{b}{qc}{h}"
                    rw = smol.tile([65, QCH], F32, tag="rw", name=f"rw{nm}")
                    nc.scalar.activation(rw[64:65, :], psO[64:65, :], AF.Copy)
                    rz = smol.tile([1, QCH], F32, tag="rz", name=f"rz{nm}")
                    nc.sync.dma_start(rz[:], rw[64:65, :])
                    rr = smol.tile([1, QCH], F32, tag="rr", name=f"rr{nm}")
                    nc.vector.reciprocal_approx_fast(rr[:], rz[:])
                    bc = bcp.tile([128, QCH], F32, tag="bc", name=f"bc{nm}")
                    nc.gpsimd.partition_broadcast(bc[:], rr[:])
                    if h == 0:
                        nc.vector.tensor_mul(OT[b][0:64, oqs].bitcast(F32R),
                                             psO[0:64, :], bc[0:64, :])
                    else:
                        otmp = bcp.tile([64, QCH], F32, tag="otmp",
                                        name=f"otmp{nm}")
                        nc.vector.tensor_mul(otmp[:], psO[0:64, :],
                                             bc[0:64, :])
                        nc.sync.dma_start(OT[b][64:128, oqs].bitcast(F32R),
                                          otmp[:].bitcast(F32R))
                if qc_done is not None:
                    qc_done(qc)

        def proj_unit_fn(psy, ybp, b):
            def unit_for(tt):
                def unit(tt=tt, b=b):
                    for ncol in range(2):
                        nm = f"{b}{tt}{ncol}"
                        ps = psy.tile([128, QCH], F32, tag="y",
                                      name=f"psy{nm}")
                        nc.tensor.matmul(
                            ps[:],
                            OT[b][:, tt * 128:(tt + 1) * 128].bitcast(F32R),
                            wo_sb[:, ncol * QCH:(ncol + 1) * QCH]
                            .bitcast(F32R),
                            start=True, stop=True)
                        yb = ybp.tile([128, QCH], F32, tag="yb",
                                      name=f"yb{nm}")
                        nc.vector.tensor_copy(yb[:], ps[:])
                        nc.sync.dma_start(
                            y[b * T + tt * 128: b * T + (tt + 1) * 128,
                              ncol * QCH:(ncol + 1) * QCH], yb[:])
                return unit
            return unit_for

        # ---- phases, per batch ----
        with tc.tile_pool(name="xp", bufs=16) as xp, \
             tc.tile_pool(name="psqkv", bufs=4, space="PSUM") as psqkv, \
             tc.tile_pool(name="pstr", bufs=2, space="PSUM") as pstr, \
             tc.tile_pool(name="swp", bufs=2) as swpp, \
             tc.tile_pool(name="rtmp", bufs=2) as rtp:
            qkv_pair(xp, psqkv, 0, 1)
            qkv_pair(xp, psqkv, 2, 3)
            rope_b(swpp, rtp, qT, 0, "q0")
            rope_b(swpp, rtp, kT, 0, "k0")
            vtrans_b(pstr, 0)
            qkv_pair(xp, psqkv, 4, 5)
            qkv_pair(xp, psqkv, 6, 7)
            rope_b(swpp, rtp, qT, 1, "q1")
            rope_b(swpp, rtp, kT, 1, "k1")
            vtrans_b(pstr, 1)

        from collections import deque
        for b in range(B):
            with tc.tile_pool(name="pss", bufs=2, space="PSUM") as pss, \
                 tc.tile_pool(name="pso", bufs=2, space="PSUM") as pso, \
                 tc.tile_pool(name="ptp", bufs=5) as ptp, \
                 tc.tile_pool(name="mbp", bufs=4) as mbp, \
                 tc.tile_pool(name="smol", bufs=4) as smol, \
                 tc.tile_pool(name="bcp", bufs=4) as bcp:
                attn_b((pss, pso, ptp, mbp, smol, bcp), b, deque())
            with tc.tile_pool(name="psy", bufs=4, space="PSUM") as psy, \
                 tc.tile_pool(name="ybp", bufs=4) as ybp:
                unit = proj_unit_fn(psy, ybp, b)
                for tt in range(NKT):
                    unit(tt)()


def _build_program(mode):
    if mode in _PROG_CACHE:
        return _PROG_CACHE[mode]
    nc = bacc.Bacc("TRN2", target_bir_lowering=False, debug=False,
                   num_devices=NCORES)
    dram = {
        "xT": nc.dram_tensor("xT", [C, TB], F32, kind="ExternalInput").ap(),
        "wT": nc.dram_tensor("wT", [C, 384], F32, kind="ExternalInput").ap(),
        "bqk": nc.dram_tensor("bqk", [128, 2], F32, kind="ExternalInput").ap(),
        "cosT": nc.dram_tensor("cosT", [128, T], F32, kind="ExternalInput").ap(),
        "sinS": nc.dram_tensor("sinS", [128, T], F32, kind="ExternalInput").ap(),
        "woT": nc.dram_tensor("woT", [128, C], F32, kind="ExternalInput").ap(),
        "y": nc.dram_tensor("y", [TB, C], F32, kind="ExternalOutput").ap(),
    }
    if mode == "bias":
        dram["maskT"] = nc.dram_tensor("maskT", [T, T], F32,
                                       kind="ExternalInput").ap()
    with tile.TileContext(nc) as tc:
        _emit(tc, mode, dram)
    nc.compile()
    _PROG_CACHE[mode] = (nc, dram)
    return nc, dram


def _rope_tables():
    inv_freq = 1.0 / (10000.0 ** (np.arange(0, HD, 2, dtype=np.float64) / HD))
    freqs = np.arange(T, dtype=np.float64)[:, None] * inv_freq[None, :]
    cos = np.concatenate([np.cos(freqs), np.cos(freqs)], axis=-1)  # [T, 64]
    sin = np.concatenate([np.sin(freqs), np.sin(freqs)], axis=-1)
    cE = cos[:, 0::2].T  # [32, T] rows i -> dim 2i
    cO = cos[:, 1::2].T
    sE = sin[:, 0::2].T
    sO = sin[:, 1::2].T
    cosT = np.concatenate([cE, cO, cE, cO], axis=0).astype(np.float32)
    sinS = np.concatenate([-sE, sO, -sE, sO], axis=0).astype(np.float32)
    return np.ascontiguousarray(cosT), np.ascontiguousarray(sinS)


def kernel(x, mask, Wqkv, bqkv, Wo, bo):
    x = np.asarray(x, dtype=np.float32)
    mask = np.asarray(mask)
    Wqkv = np.asarray(Wqkv, dtype=np.float32)
    bqkv = np.asarray(bqkv, dtype=np.float32)
    Wo = np.asarray(Wo, dtype=np.float32)
    bo = np.asarray(bo, dtype=np.float32)

    mb = mask.reshape(T, T)
    if np.array_equal(mb != 0, np.tril(np.ones((T, T), dtype=bool))):
        mode = "causal"
    elif np.all(mb != 0):
        mode = "dense"
    else:
        mode = "bias"

    nc, dram = _build_program(mode)

    xTn = np.ascontiguousarray(x.reshape(TB, C).T)
    cosT, sinS = _rope_tables()
    scale = 1.0 / np.sqrt(np.float32(HD))

    evens = np.arange(0, HD, 2)
    odds = evens + 1

    in_maps = []
    for c in range(NCORES):
        h0, h1 = 2 * c, 2 * c + 1
        qrows = np.concatenate([h0 * HD + evens, h0 * HD + odds,
                                h1 * HD + evens, h1 * HD + odds])
        krows = C + qrows
        vrows = np.concatenate([2 * C + h0 * HD + np.arange(HD),
                                2 * C + h1 * HD + np.arange(HD)])
        wq = Wqkv[qrows, :] * scale
        wk = Wqkv[krows, :]
        wv = Wqkv[vrows, :]
        wT = np.ascontiguousarray(np.concatenate([wq, wk, wv], axis=0).T)
        bqk = np.stack([bqkv[qrows] * scale, bqkv[krows]], axis=1)
        woT = np.ascontiguousarray(Wo[:, 128 * c:128 * (c + 1)].T)
        im = {
            "xT": xTn, "wT": wT,
            "bqk": np.ascontiguousarray(bqk, dtype=np.float32),
            "cosT": cosT, "sinS": sinS, "woT": woT,
        }
        if mode == "bias":
            im["maskT"] = np.ascontiguousarray(
                (mb != 0).astype(np.float32).T)
        in_maps.append(im)

    res = run_bass_kernel_spmd(nc, in_maps, core_ids=list(range(NCORES)))
    y = np.zeros((TB, C), dtype=np.float32)
    for c in range(NCORES):
        y += res.results[c]["y"]
    bv = bqkv[2 * C:3 * C]
    y += (bo + bv @ Wo.T)[None, :]
    return y.reshape(B, T, C)



# revision 2
# speedup vs baseline: 1.1058x; 1.1058x over previous
"""Trainium2 Bass kernel for CarlosSelfAttention (B=2, T=2048, C=1024, H=16).

Sharding: tensor-parallel over heads. 8 cores x 2 heads each.
Each core computes q/k/v projections for its 2 heads, RoPE, causal
attention, and a partial out-projection against its 128 columns of Wo.
The host sums the 8 partial outputs (the TP all-reduce) and adds the
output bias plus the (v-bias @ Wo.T) correction term.

v2: all matmul operands in bf16 (fp32r streams at ~2 cycles/row on HW;
bf16 is 1 cycle/row and gets fast weight load), x kept resident in
SBUF, V computed directly in [token, dim] layout by swapping matmul
roles (stationary = x tile, moving = Wv columns) so the transpose
phase disappears, causal diagonal tiles tail-sliced so masked columns
are never computed, output partials returned in bf16.

On-chip layouts are "transposed" ([dim, token]) so every matmul
contraction lands on the partition axis:
  x_sb [128, 8, 4096] bf16 (input, resident)
  qT/kT[128, 4096] bf16  rows = [h0-even dims, h0-odd, h1-even, h1-odd]
  Vsb  4 x [128, 16*65] bf16: per k-tile [128 ktok, 64 dims + ones col]
  S^T  [k-tile 128, q-cols] via PE, exp'd on ScalarE from PSUM -> bf16
  P@V  psO[65, 512] per head, sum(exp) lands in row 64 via ones col
  out  y_part [4096, 1024] bf16 = OT.T @ WoT via PE, DMA'd per 128-row
"""

import numpy as np
import ml_dtypes

import concourse.bass as bass
import concourse.tile as tile
from concourse import bacc, mybir
from concourse.bass_utils import run_bass_kernel_spmd

F32 = mybir.dt.float32
BF16 = mybir.dt.bfloat16
AF = mybir.ActivationFunctionType
NPBF16 = ml_dtypes.bfloat16

B, T, C, H, HD = 2, 2048, 1024, 16, 64
NCORES = 8
TB = B * T          # 4096
QCH = 512           # q-chunk (moving dim)
NQC = T // QCH      # 4 q-chunks per batch
NKT = T // 128      # 16 k-tiles per batch
NTC = TB // QCH     # 8 t-chunks for the projections
NCT = C // 128      # 8 contraction tiles

_PROG_CACHE: dict = {}


def _emit(tc, mode, dram):
    nc = tc.nc
    from contextlib import ExitStack

    xT, wT, bqk, cosT, sinS, woT, y = (
        dram["xT"], dram["wT"], dram["bqk"], dram["cosT"], dram["sinS"],
        dram["woT"], dram["y"])
    maskT = dram.get("maskT")

    with ExitStack() as ctx:
        constp = ctx.enter_context(tc.tile_pool(name="const", bufs=1))
        pers = ctx.enter_context(tc.tile_pool(name="pers", bufs=1))

        # ---- constants ----
        wsb = constp.tile([128, NCT, 384], BF16)
        nc.sync.dma_start(wsb[:], wT[:].rearrange("(a p) m -> p a m", p=128))
        x_sb = constp.tile([128, NCT, TB], BF16)
        for ct in range(NCT):
            nc.sync.dma_start(x_sb[:, ct, :],
                              xT[ct * 128:(ct + 1) * 128, :])
        cos_sb = constp.tile([128, T], BF16)
        nc.sync.dma_start(cos_sb[:], cosT[:])
        sin_sb = constp.tile([128, T], BF16)
        nc.sync.dma_start(sin_sb[:], sinS[:])
        bqk_sb = constp.tile([128, 2], F32)
        nc.sync.dma_start(bqk_sb[:], bqk[:])
        wo_sb = constp.tile([128, C], BF16)
        nc.sync.dma_start(wo_sb[:], woT[:])

        # ---- persistent activations ----
        qT = pers.tile([128, TB], BF16)
        kT = pers.tile([128, TB], BF16)
        # V in [ktok, dim] layout; col 64 of each k-tile stays 1.0 so the
        # P@V matmul also produces sum(exp) in psO row 64.
        Vsb = [[pers.tile([128, NKT * (HD + 1)], BF16, name=f"Vsb{b}{h}")
                for h in range(2)] for b in range(B)]
        for b in range(B):
            for h in range(2):
                nc.vector.memset(Vsb[b][h][:], 1.0)
        OT = [pers.tile([128, T], BF16, name=f"OTb{b}") for b in range(B)]

        def qk_pair(xp, psqkv, tca, tcb):
            """q and k projections for two t-chunks, shared stationaries."""
            tsa = slice(tca * QCH, (tca + 1) * QCH)
            tsb = slice(tcb * QCH, (tcb + 1) * QCH)
            for g in range(2):
                psa = psqkv.tile([128, QCH], F32, tag="ps",
                                 name=f"psq{tca}_{g}")
                psb = psqkv.tile([128, QCH], F32, tag="ps",
                                 name=f"psq{tcb}_{g}")
                for ct in range(NCT):
                    w = wsb[:, ct, g * 128:(g + 1) * 128]
                    nc.tensor.matmul(psa[:], w, x_sb[:, ct, tsa],
                                     start=(ct == 0), stop=(ct == NCT - 1))
                    nc.tensor.matmul(psb[:], w, x_sb[:, ct, tsb],
                                     start=(ct == 0), stop=(ct == NCT - 1))
                dst = qT if g == 0 else kT
                nc.scalar.activation(dst[:, tsa], psa[:],
                                     AF.Identity, bias=bqk_sb[:, g:g + 1])
                nc.scalar.activation(dst[:, tsb], psb[:],
                                     AF.Identity, bias=bqk_sb[:, g:g + 1])

        def v_tiles(psvp, b):
            """V for batch b directly in [ktok, dim] layout: stationary is
            the x tile, moving is the 128 Wv columns (both heads)."""
            for tt in range(NKT):
                ts = slice(b * T + tt * 128, b * T + (tt + 1) * 128)
                psv = psvp.tile([128, 128], F32, tag="v", name=f"psv{b}{tt}")
                for ct in range(NCT):
                    nc.tensor.matmul(psv[:], x_sb[:, ct, ts],
                                     wsb[:, ct, 256:384],
                                     start=(ct == 0), stop=(ct == NCT - 1))
                o = tt * (HD + 1)
                nc.vector.tensor_copy(Vsb[b][0][:, o:o + HD], psv[:, 0:HD])
                nc.vector.tensor_copy(Vsb[b][1][:, o:o + HD],
                                      psv[:, HD:2 * HD])

        def rope_b(swpp, rtp, zt, b, nm):
            bs = slice(b * T, (b + 1) * T)
            swp = swpp.tile([128, T], BF16, tag="swp", name=f"swp{nm}")
            for h in range(2):
                o = h * 64
                nc.sync.dma_start(swp[o:o + 32, :], zt[o + 32:o + 64, bs])
                nc.sync.dma_start(swp[o + 32:o + 64, :], zt[o:o + 32, bs])
            tmp = rtp.tile([128, T], BF16, tag="rt", name=f"rt{nm}")
            nc.vector.tensor_mul(tmp[:], swp[:], sin_sb[:])
            nc.vector.tensor_mul(zt[:, bs], zt[:, bs], cos_sb[:])
            nc.vector.tensor_add(zt[:, bs], zt[:, bs], tmp[:])

        def attn_b(pools, b):
            pss, pso, ptp, mbp, smol, bcp = pools
            PIPE = 3
            for qc in range(NQC):
                nk = 4 * (qc + 1) if mode == "causal" else NKT
                psO = [pso.tile([65, QCH], F32, tag=f"o{h}",
                                name=f"psO{h}_{b}{qc}") for h in range(2)]
                pts = {}

                def offs(kt):
                    if mode == "causal" and kt >= 4 * qc:
                        return (kt - 4 * qc) * 128
                    return 0

                def emit_pv(j, nk=nk, psO=psO, pts=pts):
                    st, sp = (j == 0), (j == nk - 1)
                    off = offs(j)
                    pt = pts.pop(j)
                    for h in range(2):
                        nc.tensor.matmul(
                            psO[h][:, off:QCH],
                            Vsb[b][h][:, j * (HD + 1):(j + 1) * (HD + 1)],
                            pt[h][:, off:QCH], start=st, stop=sp)

                for kt in range(nk):
                    ks = slice(b * T + kt * 128, b * T + (kt + 1) * 128)
                    off = offs(kt)
                    qs = slice(b * T + qc * QCH + off, b * T + (qc + 1) * QCH)
                    pt = []
                    for h in range(2):
                        hp = slice(h * 64, (h + 1) * 64)
                        psS = pss.tile([128, QCH], F32, tag="s",
                                       name=f"psS{h}_{b}{qc}{kt}")
                        nc.tensor.matmul(psS[:, off:QCH], kT[hp, ks],
                                         qT[hp, qs], start=True, stop=True)
                        pth = ptp.tile([128, QCH], BF16, tag="pt",
                                       name=f"pt{h}_{b}{qc}{kt}")
                        nc.scalar.activation(pth[:, off:QCH], psS[:, off:QCH],
                                             AF.Exp)
                        if mode == "causal" and kt >= 4 * qc:
                            nc.gpsimd.affine_select(
                                out=pth[:, off:off + 128],
                                in_=pth[:, off:off + 128],
                                compare_op=mybir.AluOpType.is_ge,
                                fill=0.0, base=0, channel_multiplier=-1,
                                pattern=[[1, 128]])
                        elif mode == "bias":
                            mt = mbp.tile([128, QCH], BF16, tag="mb",
                                          name=f"mt{h}_{b}{qc}{kt}")
                            nc.sync.dma_start(
                                mt[:], maskT[kt * 128:(kt + 1) * 128,
                                             qc * QCH:(qc + 1) * QCH])
                            nc.vector.tensor_mul(pth[:], pth[:], mt[:])
                        pt.append(pth)
                    pts[kt] = pt
                    if kt >= PIPE:
                        emit_pv(kt - PIPE)
                for j in range(max(0, nk - PIPE), nk):
                    emit_pv(j)

                # normalize + evict; sum(exp) sits in row 64 of psO[h]
                oqs = slice(qc * QCH, (qc + 1) * QCH)
                for h in range(2):
                    nm = f"{b}{qc}{h}"
                    rw = smol.tile([65, QCH], F32, tag="rw", name=f"rw{nm}")
                    nc.scalar.activation(rw[64:65, :], psO[h][64:65, :],
                                         AF.Copy)
                    rz = smol.tile([1, QCH], F32, tag="rz", name=f"rz{nm}")
                    nc.sync.dma_start(rz[:], rw[64:65, :])
                    rr = smol.tile([1, QCH], F32, tag="rr", name=f"rr{nm}")
                    nc.vector.reciprocal_approx_fast(rr[:], rz[:])
                    bc = bcp.tile([64, QCH], F32, tag="bc", name=f"bc{nm}")
                    nc.gpsimd.partition_broadcast(bc[:], rr[:])
                    if h == 0:
                        nc.vector.tensor_mul(OT[b][0:64, oqs],
                                             psO[h][0:64, :], bc[:])
                    else:
                        otmp = bcp.tile([64, QCH], BF16, tag="otmp",
                                        name=f"otmp{nm}")
                        nc.vector.tensor_mul(otmp[:], psO[h][0:64, :], bc[:])
                        nc.sync.dma_start(OT[b][64:128, oqs], otmp[:])

        def proj_b(psy, ybp, b):
            for tt in range(NKT):
                yb = ybp.tile([128, C], BF16, tag="yb", name=f"yb{b}{tt}")
                for ncol in range(2):
                    ps = psy.tile([128, QCH], F32, tag="y",
                                  name=f"psy{b}{tt}{ncol}")
                    nc.tensor.matmul(
                        ps[:], OT[b][:, tt * 128:(tt + 1) * 128],
                        wo_sb[:, ncol * QCH:(ncol + 1) * QCH],
                        start=True, stop=True)
                    nc.vector.tensor_copy(yb[:, ncol * QCH:(ncol + 1) * QCH],
                                          ps[:])
                nc.sync.dma_start(
                    y[b * T + tt * 128:b * T + (tt + 1) * 128, :], yb[:])

        # ---- phases ----
        with tc.tile_pool(name="psqkv", bufs=4, space="PSUM") as psqkv, \
             tc.tile_pool(name="psv", bufs=4, space="PSUM") as psv, \
             tc.tile_pool(name="swp", bufs=2) as swpp, \
             tc.tile_pool(name="rtmp", bufs=2) as rtp:
            for b in range(B):
                qk_pair(None, psqkv, 4 * b + 0, 4 * b + 1)
                qk_pair(None, psqkv, 4 * b + 2, 4 * b + 3)
                v_tiles(psv, b)
                rope_b(swpp, rtp, qT, b, f"q{b}")
                rope_b(swpp, rtp, kT, b, f"k{b}")

        with tc.tile_pool(name="pss", bufs=4, space="PSUM") as pss, \
             tc.tile_pool(name="pso", bufs=2, space="PSUM") as pso, \
             tc.tile_pool(name="ptp", bufs=8) as ptp, \
             tc.tile_pool(name="mbp", bufs=4) as mbp, \
             tc.tile_pool(name="smol", bufs=4) as smol, \
             tc.tile_pool(name="bcp", bufs=4) as bcp:
            for b in range(B):
                attn_b((pss, pso, ptp, mbp, smol, bcp), b)
        with tc.tile_pool(name="psy", bufs=4, space="PSUM") as psy, \
             tc.tile_pool(name="ybp", bufs=4) as ybp:
            for b in range(B):
                proj_b(psy, ybp, b)


def _build_program(mode):
    if mode in _PROG_CACHE:
        return _PROG_CACHE[mode]
    nc = bacc.Bacc("TRN2", target_bir_lowering=False, debug=False,
                   num_devices=NCORES)
    dram = {
        "xT": nc.dram_tensor("xT", [C, TB], BF16, kind="ExternalInput").ap(),
        "wT": nc.dram_tensor("wT", [C, 384], BF16, kind="ExternalInput").ap(),
        "bqk": nc.dram_tensor("bqk", [128, 2], F32, kind="ExternalInput").ap(),
        "cosT": nc.dram_tensor("cosT", [128, T], BF16,
                               kind="ExternalInput").ap(),
        "sinS": nc.dram_tensor("sinS", [128, T], BF16,
                               kind="ExternalInput").ap(),
        "woT": nc.dram_tensor("woT", [128, C], BF16,
                              kind="ExternalInput").ap(),
        "y": nc.dram_tensor("y", [TB, C], BF16, kind="ExternalOutput").ap(),
    }
    if mode == "bias":
        dram["maskT"] = nc.dram_tensor("maskT", [T, T], BF16,
                                       kind="ExternalInput").ap()
    with tile.TileContext(nc) as tc:
        _emit(tc, mode, dram)
    nc.compile()
    _PROG_CACHE[mode] = (nc, dram)
    return nc, dram


def _rope_tables():
    inv_freq = 1.0 / (10000.0 ** (np.arange(0, HD, 2, dtype=np.float64) / HD))
    freqs = np.arange(T, dtype=np.float64)[:, None] * inv_freq[None, :]
    cos = np.concatenate([np.cos(freqs), np.cos(freqs)], axis=-1)  # [T, 64]
    sin = np.concatenate([np.sin(freqs), np.sin(freqs)], axis=-1)
    cE = cos[:, 0::2].T  # [32, T] rows i -> dim 2i
    cO = cos[:, 1::2].T
    sE = sin[:, 0::2].T
    sO = sin[:, 1::2].T
    cosT = np.concatenate([cE, cO, cE, cO], axis=0)
    sinS = np.concatenate([-sE, sO, -sE, sO], axis=0)
    return (np.ascontiguousarray(cosT.astype(NPBF16)),
            np.ascontiguousarray(sinS.astype(NPBF16)))


def _detect_mode(mask):
    mb = mask.reshape(T, T)
    if np.array_equal(mb != 0, np.tril(np.ones((T, T), dtype=bool))):
        return "causal"
    if np.all(mb != 0):
        return "dense"
    return "bias"


def _prepare_in_maps(x, mask, Wqkv, bqkv, Wo, mode):
    xTn = np.ascontiguousarray(
        x.reshape(TB, C).T.astype(NPBF16))
    cosT, sinS = _rope_tables()
    scale = 1.0 / np.sqrt(np.float32(HD))

    evens = np.arange(0, HD, 2)
    odds = evens + 1

    in_maps = []
    for c in range(NCORES):
        h0, h1 = 2 * c, 2 * c + 1
        qrows = np.concatenate([h0 * HD + evens, h0 * HD + odds,
                                h1 * HD + evens, h1 * HD + odds])
        krows = C + qrows
        vrows = np.concatenate([2 * C + h0 * HD + np.arange(HD),
                                2 * C + h1 * HD + np.arange(HD)])
        wq = Wqkv[qrows, :] * scale
        wk = Wqkv[krows, :]
        wv = Wqkv[vrows, :]
        wT = np.ascontiguousarray(
            np.concatenate([wq, wk, wv], axis=0).T.astype(NPBF16))
        bqk = np.stack([bqkv[qrows] * scale, bqkv[krows]], axis=1)
        woT = np.ascontiguousarray(
            Wo[:, 128 * c:128 * (c + 1)].T.astype(NPBF16))
        im = {
            "xT": xTn, "wT": wT,
            "bqk": np.ascontiguousarray(bqk, dtype=np.float32),
            "cosT": cosT, "sinS": sinS, "woT": woT,
        }
        if mode == "bias":
            im["maskT"] = np.ascontiguousarray(
                (mask.reshape(T, T) != 0).T.astype(NPBF16))
        in_maps.append(im)
    return in_maps


def kernel(x, mask, Wqkv, bqkv, Wo, bo):
    x = np.asarray(x, dtype=np.float32)
    mask = np.asarray(mask)
    Wqkv = np.asarray(Wqkv, dtype=np.float32)
    bqkv = np.asarray(bqkv, dtype=np.float32)
    Wo = np.asarray(Wo, dtype=np.float32)
    bo = np.asarray(bo, dtype=np.float32)

    mode = _detect_mode(mask)
    nc, dram = _build_program(mode)
    in_maps = _prepare_in_maps(x, mask, Wqkv, bqkv, Wo, mode)

    res = run_bass_kernel_spmd(nc, in_maps, core_ids=list(range(NCORES)))
    y = np.zeros((TB, C), dtype=np.float32)
    for c in range(NCORES):
        y += res.results[c]["y"].astype(np.float32)
    bv = bqkv[2 * C:3 * C]
    y += (bo + bv @ Wo.T)[None, :]
    return y.reshape(B, T, C)


# revision 6
# speedup vs baseline: 1.1395x; 1.0306x over previous
"""Trainium2 Bass kernel for CarlosSelfAttention (B=2, T=2048, C=1024, H=16).

Sharding: tensor-parallel over heads. 8 cores x 2 heads each.
Each core computes q/k/v projections for its 2 heads, RoPE, causal
attention, and a partial out-projection against its 128 columns of Wo.
The host sums the 8 partial outputs (the TP all-reduce) and adds the
output bias plus the (v-bias @ Wo.T) correction term.

v3 highlights (the PE duty-cycles throttle under sustained load, so
total PE cycles are the budget):
  - all matmul operands bf16 (fp32r streams at ~2 cyc/row on HW)
  - the two heads' score matmuls sit on disjoint PE row groups
    (partitions 0:64 / 64:128) and are emitted back-to-back so they
    run concurrently -> score cycles halved
  - causal diagonal tiles tail-sliced: masked columns never computed
  - one fused exp per k-tile (both heads) to amortize the ~352-cycle
    ACTIVATE startup; causal masking via a precomputed triangular
    bf16 tile multiplied on the vector engine
  - V computed directly in [token, dim] layout (stationary = x tile,
    moving = Wv columns) so there is no transpose phase
  - out-projection interleaved into the attention loop per q-chunk,
    PSUM shared with the psO pool; evictions spread over
    Vector/Scalar/GpSimd to keep every engine under the PE wall
"""

import numpy as np
import ml_dtypes

import concourse.bass as bass
import concourse.tile as tile
from concourse import bacc, mybir
from concourse.bass_utils import run_bass_kernel_spmd

F32 = mybir.dt.float32
BF16 = mybir.dt.bfloat16
AF = mybir.ActivationFunctionType
NPBF16 = ml_dtypes.bfloat16

B, T, C, H, HD = 2, 2048, 1024, 16, 64
NCORES = 8
TB = B * T          # 4096
QCH = 512           # q-chunk (moving dim)
NQC = T // QCH      # 4 q-chunks per batch
NKT = T // 128      # 16 k-tiles per batch
NCT = C // 128      # 8 contraction tiles

_PROG_CACHE: dict = {}


def _emit(tc, mode, hasb, dram):
    nc = tc.nc
    from contextlib import ExitStack

    wT, bqk, cosT, sinS, woT, y = (
        dram["wT"], dram["bqk"], dram["cosT"], dram["sinS"],
        dram["woT"], dram["y"])
    xT = dram["xT"]
    maskT = dram.get("maskT")

    with ExitStack() as ctx:
        constp = ctx.enter_context(tc.tile_pool(name="const", bufs=1))
        pers = ctx.enter_context(tc.tile_pool(name="pers", bufs=1))

        # ---- constants (wsb + first x quarter first: qkv b0 needs them) ----
        wsb = constp.tile([128, NCT, 384], BF16)
        nc.sync.dma_start(wsb[:], wT[:].rearrange("(a p) m -> p a m", p=128))
        xq = [constp.tile([128, NCT, 1024], BF16, name=f"xq{i}")
              for i in range(4)]

        def xs(ct, lo, n):
            """x slice [128, n] covering global token cols [lo, lo+n)."""
            q, l = lo // 1024, lo % 1024
            assert l + n <= 1024
            return xq[q][:, ct, l:l + n]

        nc.sync.dma_start(
            xq[0][:], xT[:, 0:1024].rearrange("(a p) m -> p a m", p=128))
        nc.sync.dma_start(
            xq[1][:], xT[:, 1024:2048].rearrange("(a p) m -> p a m", p=128))
        cos_sb = constp.tile([128, T], BF16)
        nc.sync.dma_start(cos_sb[:], cosT[:])
        sin_sb = constp.tile([128, T], BF16)
        nc.sync.dma_start(sin_sb[:], sinS[:])
        bqk_sb = constp.tile([128, 2], F32)
        nc.sync.dma_start(bqk_sb[:], bqk[:])
        wo_sb = constp.tile([128, C], BF16)
        nc.sync.dma_start(wo_sb[:], woT[:])
        nc.sync.dma_start(
            xq[2][:], xT[:, 2048:3072].rearrange("(a p) m -> p a m", p=128))
        nc.sync.dma_start(
            xq[3][:], xT[:, 3072:4096].rearrange("(a p) m -> p a m", p=128))
        # triangular causal mask for the diagonal 128-blocks, both heads:
        # [128, 2, 128], keep (1.0) iff col >= partition
        mtri = constp.tile([128, 2, 128], BF16)
        nc.vector.memset(mtri[:], 1.0)
        nc.gpsimd.affine_select(
            out=mtri[:], in_=mtri[:], compare_op=mybir.AluOpType.is_ge,
            fill=0.0, base=0, channel_multiplier=-1,
            pattern=[[0, 2], [1, 128]])

        # ---- persistent activations ----
        qT = pers.tile([128, TB], BF16)
        kT = pers.tile([128, TB], BF16)
        # V in [ktok, dim] layout; col 64 of each k-tile stays 1.0 so the
        # P@V matmul also produces sum(exp) in psO row 64.
        Vsb = [[pers.tile([128, NKT * (HD + 1)], BF16, name=f"Vsb{b}{h}")
                for h in range(2)] for b in range(B)]
        for b in range(B):
            for h in range(2):
                nc.vector.memset(Vsb[b][h][:], 1.0)
        OT = [pers.tile([128, T], BF16, name=f"OTb{b}") for b in range(B)]

        def qk_pair(psqkv, tca, tcb):
            """q and k projections for two 512-token chunks, shared
            stationaries."""
            tsa = slice(tca * QCH, (tca + 1) * QCH)
            tsb = slice(tcb * QCH, (tcb + 1) * QCH)
            for g in range(2):
                psa = psqkv.tile([128, QCH], F32, tag="ps",
                                 name=f"psq{tca}_{g}")
                psb = psqkv.tile([128, QCH], F32, tag="ps",
                                 name=f"psq{tcb}_{g}")
                for ct in range(NCT):
                    w = wsb[:, ct, g * 128:(g + 1) * 128]
                    nc.tensor.matmul(psa[:], w, xs(ct, tca * QCH, QCH),
                                     start=(ct == 0), stop=(ct == NCT - 1))
                    nc.tensor.matmul(psb[:], w, xs(ct, tcb * QCH, QCH),
                                     start=(ct == 0), stop=(ct == NCT - 1))
                dst = qT if g == 0 else kT
                if hasb:
                    nc.scalar.activation(dst[:, tsa], psa[:], AF.Identity,
                                         bias=bqk_sb[:, g:g + 1])
                    nc.scalar.activation(dst[:, tsb], psb[:], AF.Identity,
                                         bias=bqk_sb[:, g:g + 1])
                else:
                    nc.vector.tensor_copy(dst[:, tsa], psa[:])
                    nc.vector.tensor_copy(dst[:, tsb], psb[:])

        def v_tiles(psvp, b):
            """V for batch b directly in [ktok, dim] layout: stationary is
            the x tile, moving is the 128 Wv columns (both heads)."""
            for tt in range(NKT):
                lo = b * T + tt * 128
                psv = psvp.tile([128, 128], F32, tag="v", name=f"psv{b}{tt}")
                for ct in range(NCT):
                    nc.tensor.matmul(psv[:], xs(ct, lo, 128),
                                     wsb[:, ct, 256:384],
                                     start=(ct == 0), stop=(ct == NCT - 1))
                o = tt * (HD + 1)
                nc.vector.tensor_copy(Vsb[b][0][:, o:o + HD], psv[:, 0:HD])
                nc.vector.tensor_copy(Vsb[b][1][:, o:o + HD],
                                      psv[:, HD:2 * HD])

        def rope_b(swpp, rtp, zt, b, nm):
            bs = slice(b * T, (b + 1) * T)
            swp = swpp.tile([128, T], BF16, tag="swp", name=f"swp{nm}")
            for h in range(2):
                o = h * 64
                nc.sync.dma_start(swp[o:o + 32, :], zt[o + 32:o + 64, bs])
                nc.sync.dma_start(swp[o + 32:o + 64, :], zt[o:o + 32, bs])
            tmp = rtp.tile([128, T], BF16, tag="rt", name=f"rt{nm}")
            nc.vector.tensor_mul(tmp[:], swp[:], sin_sb[:])
            nc.vector.tensor_mul(zt[:, bs], zt[:, bs], cos_sb[:])
            nc.vector.tensor_add(zt[:, bs], zt[:, bs], tmp[:])

        def attn_b(pools, b, ybn):
            pss, pso, ptp, mbp, smol = pools
            PIPE = 3
            for qc in range(NQC):
                nk = 4 * (qc + 1) if mode == "causal" else NKT
                psO = [pso.tile([65, QCH], F32, tag=f"o{h}",
                                name=f"psO{h}_{b}{qc}") for h in range(2)]
                pts = {}

                def offs(kt):
                    if mode == "causal" and kt >= 4 * qc:
                        return (kt - 4 * qc) * 128
                    return 0

                def emit_pv(j, nk=nk, psO=psO, pts=pts):
                    st, sp = (j == 0), (j == nk - 1)
                    off = offs(j)
                    pt = pts.pop(j)
                    ptv = pt[:].rearrange("p (h q) -> p h q", q=QCH)
                    for h in range(2):
                        nc.tensor.matmul(
                            psO[h][:, off:QCH],
                            Vsb[b][h][:, j * (HD + 1):(j + 1) * (HD + 1)],
                            ptv[:, h, off:QCH], start=st, stop=sp)

                for kt in range(nk):
                    ks = slice(b * T + kt * 128, b * T + (kt + 1) * 128)
                    off = offs(kt)
                    qs = slice(b * T + qc * QCH + off, b * T + (qc + 1) * QCH)
                    psS = pss.tile([128, 2 * QCH], F32, tag="s",
                                   name=f"psS{b}{qc}{kt}")
                    # the two heads occupy disjoint PE row groups
                    # (partitions 0:64 / 64:128) -> emitted back-to-back
                    # they execute concurrently
                    nc.tensor.matmul(psS[:, off:QCH], kT[0:64, ks],
                                     qT[0:64, qs], start=True, stop=True)
                    nc.tensor.matmul(psS[:, QCH + off:2 * QCH],
                                     kT[64:128, ks], qT[64:128, qs],
                                     start=True, stop=True)
                    pt = ptp.tile([128, 2 * QCH], BF16, tag="pt",
                                  name=f"pt{b}{qc}{kt}")
                    psv_ = psS[:].rearrange("p (h q) -> p h q", q=QCH)
                    ptv = pt[:].rearrange("p (h q) -> p h q", q=QCH)
                    nc.scalar.activation(ptv[:, :, off:QCH],
                                         psv_[:, :, off:QCH], AF.Exp)
                    if mode == "causal" and kt >= 4 * qc:
                        nc.vector.tensor_mul(ptv[:, :, off:off + 128],
                                             ptv[:, :, off:off + 128],
                                             mtri[:])
                    elif mode == "bias":
                        mt = mbp.tile([128, QCH], BF16, tag="mb",
                                      name=f"mt{b}{qc}{kt}")
                        nc.sync.dma_start(
                            mt[:], maskT[kt * 128:(kt + 1) * 128,
                                         qc * QCH:(qc + 1) * QCH])
                        nc.vector.tensor_mul(ptv[:, 0, :], ptv[:, 0, :],
                                             mt[:])
                        nc.vector.tensor_mul(ptv[:, 1, :], ptv[:, 1, :],
                                             mt[:])
                    pts[kt] = pt
                    if kt >= PIPE:
                        emit_pv(kt - PIPE)
                for j in range(max(0, nk - PIPE), nk):
                    emit_pv(j)

                # normalize + evict; sum(exp) sits in row 64 of psO[h]
                oqs = slice(qc * QCH, (qc + 1) * QCH)
                for h in range(2):
                    nm = f"{b}{qc}{h}"
                    rw = smol.tile([65, QCH], F32, tag="rw", name=f"rw{nm}")
                    nc.vector.tensor_copy(rw[64:65, :], psO[h][64:65, :])
                    rz = smol.tile([1, QCH], F32, tag="rz", name=f"rz{nm}")
                    nc.sync.dma_start(rz[:], rw[64:65, :])
                    rr = smol.tile([1, QCH], F32, tag="rr", name=f"rr{nm}")
                    nc.vector.reciprocal_approx_fast(rr[:], rz[:])
                    bc = smol.tile([64, QCH], F32, tag="bc", name=f"bc{nm}")
                    nc.gpsimd.partition_broadcast(bc[:], rr[:])
                    if h == 0:
                        nc.vector.tensor_mul(OT[b][0:64, oqs],
                                             psO[h][0:64, :], bc[:])
                    else:
                        otmp = smol.tile([64, QCH], BF16, tag="otmp",
                                         name=f"otmp{nm}")
                        nc.vector.tensor_mul(otmp[:], psO[h][0:64, :], bc[:])
                        nc.sync.dma_start(OT[b][64:128, oqs], otmp[:])

                # out-projection for the 4 token-tiles this q-chunk
                # completed; psy shares the pss pool's PSUM slots (tag "s")
                for tt in range(4 * qc, 4 * qc + 4):
                    yb = ybn.tile([128, C], BF16, tag="yb", name=f"yb{b}{tt}")
                    ps = pss.tile([128, 2 * QCH], F32, tag="s",
                                  name=f"psy{b}{tt}")
                    for ncol in range(2):
                        nc.tensor.matmul(
                            ps[:, ncol * QCH:(ncol + 1) * QCH],
                            OT[b][:, tt * 128:(tt + 1) * 128],
                            wo_sb[:, ncol * QCH:(ncol + 1) * QCH],
                            start=True, stop=True)
                    if tt % 2 == 0:
                        nc.vector.tensor_copy(yb[:], ps[:])
                    else:
                        nc.scalar.activation(yb[:], ps[:], AF.Copy)
                    nc.sync.dma_start(
                        y[b * T + tt * 128:b * T + (tt + 1) * 128, :], yb[:])

        # ---- phases ----
        with tc.tile_pool(name="psqkv", bufs=4, space="PSUM") as psqkv, \
             tc.tile_pool(name="psv", bufs=4, space="PSUM") as psvp, \
             tc.tile_pool(name="swp", bufs=2) as swpp, \
             tc.tile_pool(name="rtmp", bufs=2) as rtp:
            for b in range(B):
                qk_pair(psqkv, 4 * b + 0, 4 * b + 1)
                qk_pair(psqkv, 4 * b + 2, 4 * b + 3)
                v_tiles(psvp, b)
                rope_b(swpp, rtp, qT, b, f"q{b}")
                rope_b(swpp, rtp, kT, b, f"k{b}")

        with tc.tile_pool(name="pss", bufs=2, space="PSUM") as pss, \
             tc.tile_pool(name="pso", bufs=2, space="PSUM") as pso, \
             tc.tile_pool(name="ptp", bufs=4) as ptp, \
             tc.tile_pool(name="mbp", bufs=4) as mbp, \
             tc.tile_pool(name="smol", bufs=4) as smol, \
             tc.tile_pool(name="ybp", bufs=4) as ybp:
            for b in range(B):
                attn_b((pss, pso, ptp, mbp, smol), b, ybp)


def _build_program(mode, hasb):
    key = (mode, hasb)
    if key in _PROG_CACHE:
        return _PROG_CACHE[key]
    nc = bacc.Bacc("TRN2", target_bir_lowering=False, debug=False,
                   num_devices=NCORES)
    dram = {
        "xT": nc.dram_tensor("xT", [C, TB], BF16, kind="ExternalInput").ap(),
        "wT": nc.dram_tensor("wT", [C, 384], BF16, kind="ExternalInput").ap(),
        "bqk": nc.dram_tensor("bqk", [128, 2], F32, kind="ExternalInput").ap(),
        "cosT": nc.dram_tensor("cosT", [128, T], BF16,
                               kind="ExternalInput").ap(),
        "sinS": nc.dram_tensor("sinS", [128, T], BF16,
                               kind="ExternalInput").ap(),
        "woT": nc.dram_tensor("woT", [128, C], BF16,
                              kind="ExternalInput").ap(),
        "y": nc.dram_tensor("y", [TB, C], BF16, kind="ExternalOutput").ap(),
    }
    if mode == "bias":
        dram["maskT"] = nc.dram_tensor("maskT", [T, T], BF16,
                                       kind="ExternalInput").ap()
    with tile.TileContext(nc) as tc:
        _emit(tc, mode, hasb, dram)
    nc.compile()
    _PROG_CACHE[key] = (nc, dram)
    return nc, dram


def _rope_tables():
    inv_freq = 1.0 / (10000.0 ** (np.arange(0, HD, 2, dtype=np.float64) / HD))
    freqs = np.arange(T, dtype=np.float64)[:, None] * inv_freq[None, :]
    cos = np.concatenate([np.cos(freqs), np.cos(freqs)], axis=-1)  # [T, 64]
    sin = np.concatenate([np.sin(freqs), np.sin(freqs)], axis=-1)
    cE = cos[:, 0::2].T  # [32, T] rows i -> dim 2i
    cO = cos[:, 1::2].T
    sE = sin[:, 0::2].T
    sO = sin[:, 1::2].T
    cosT = np.concatenate([cE, cO, cE, cO], axis=0)
    sinS = np.concatenate([-sE, sO, -sE, sO], axis=0)
    return (np.ascontiguousarray(cosT.astype(NPBF16)),
            np.ascontiguousarray(sinS.astype(NPBF16)))


def _detect_mode(mask):
    mb = mask.reshape(T, T)
    if np.array_equal(mb != 0, np.tril(np.ones((T, T), dtype=bool))):
        return "causal"
    if np.all(mb != 0):
        return "dense"
    return "bias"


def _prepare_in_maps(x, mask, Wqkv, bqkv, Wo, mode):
    xTn = np.ascontiguousarray(
        x.reshape(TB, C).T.astype(NPBF16))
    cosT, sinS = _rope_tables()
    scale = 1.0 / np.sqrt(np.float32(HD))

    evens = np.arange(0, HD, 2)
    odds = evens + 1

    in_maps = []
    for c in range(NCORES):
        h0, h1 = 2 * c, 2 * c + 1
        qrows = np.concatenate([h0 * HD + evens, h0 * HD + odds,
                                h1 * HD + evens, h1 * HD + odds])
        krows = C + qrows
        vrows = np.concatenate([2 * C + h0 * HD + np.arange(HD),
                                2 * C + h1 * HD + np.arange(HD)])
        wq = Wqkv[qrows, :] * scale
        wk = Wqkv[krows, :]
        wv = Wqkv[vrows, :]
        wT = np.ascontiguousarray(
            np.concatenate([wq, wk, wv], axis=0).T.astype(NPBF16))
        bqk = np.stack([bqkv[qrows] * scale, bqkv[krows]], axis=1)
        woT = np.ascontiguousarray(
            Wo[:, 128 * c:128 * (c + 1)].T.astype(NPBF16))
        im = {
            "xT": xTn, "wT": wT,
            "bqk": np.ascontiguousarray(bqk, dtype=np.float32),
            "cosT": cosT, "sinS": sinS, "woT": woT,
        }
        if mode == "bias":
            im["maskT"] = np.ascontiguousarray(
                (mask.reshape(T, T) != 0).T.astype(NPBF16))
        in_maps.append(im)
    return in_maps


def kernel(x, mask, Wqkv, bqkv, Wo, bo):
    x = np.asarray(x, dtype=np.float32)
    mask = np.asarray(mask)
    Wqkv = np.asarray(Wqkv, dtype=np.float32)
    bqkv = np.asarray(bqkv, dtype=np.float32)
    Wo = np.asarray(Wo, dtype=np.float32)
    bo = np.asarray(bo, dtype=np.float32)

    mode = _detect_mode(mask)
    hasb = bool(np.any(bqkv[:2 * C] != 0.0))
    nc, dram = _build_program(mode, hasb)
    in_maps = _prepare_in_maps(x, mask, Wqkv, bqkv, Wo, mode)

    res = run_bass_kernel_spmd(nc, in_maps, core_ids=list(range(NCORES)))
    y = np.zeros((TB, C), dtype=np.float32)
    for c in range(NCORES):
        y += res.results[c]["y"].astype(np.float32)
    bv = bqkv[2 * C:3 * C]
    y += (bo + bv @ Wo.T)[None, :]
    return y.reshape(B, T, C)


# revision 10
# speedup vs baseline: 1.1739x; 1.0301x over previous
"""Trainium2 Bass kernel for CarlosSelfAttention (B=2, T=2048, C=1024, H=16).

Sharding: tensor-parallel over heads. 8 cores x 2 heads each.
Each core computes q/k/v projections for its 2 heads, RoPE, causal
attention, and a partial out-projection against its 128 columns of Wo.
The host sums the 8 partial outputs (the TP all-reduce) and adds the
output bias plus the (v-bias @ Wo.T) correction term.

v3 highlights (the PE duty-cycles throttle under sustained load, so
total PE cycles are the budget):
  - all matmul operands bf16 (fp32r streams at ~2 cyc/row on HW)
  - the two heads' score matmuls sit on disjoint PE row groups
    (partitions 0:64 / 64:128) and are emitted back-to-back so they
    run concurrently -> score cycles halved
  - causal diagonal tiles tail-sliced: masked columns never computed
  - one fused exp per k-tile (both heads) to amortize the ~352-cycle
    ACTIVATE startup; causal masking via a precomputed triangular
    bf16 tile multiplied on the vector engine
  - V computed directly in [token, dim] layout (stationary = x tile,
    moving = Wv columns) so there is no transpose phase
  - out-projection interleaved into the attention loop per q-chunk,
    PSUM shared with the psO pool; evictions spread over
    Vector/Scalar/GpSimd to keep every engine under the PE wall
"""

import numpy as np
import ml_dtypes

import concourse.bass as bass
import concourse.tile as tile
from concourse import bacc, mybir
from concourse.bass_utils import run_bass_kernel_spmd

F32 = mybir.dt.float32
BF16 = mybir.dt.bfloat16
AF = mybir.ActivationFunctionType
NPBF16 = ml_dtypes.bfloat16

B, T, C, H, HD = 2, 2048, 1024, 16, 64
NCORES = 8
TB = B * T          # 4096
QCH = 512           # q-chunk (moving dim)
NQC = T // QCH      # 4 q-chunks per batch
NKT = T // 128      # 16 k-tiles per batch
NCT = C // 128      # 8 contraction tiles

_PROG_CACHE: dict = {}


def _emit(tc, mode, hasb, dram):
    nc = tc.nc
    from contextlib import ExitStack

    wT, bqk, cosT, sinS, woT, y = (
        dram["wT"], dram["bqk"], dram["cosT"], dram["sinS"],
        dram["woT"], dram["y"])
    xT = dram["xT"]
    maskT = dram.get("maskT")

    with ExitStack() as ctx:
        constp = ctx.enter_context(tc.tile_pool(name="const", bufs=1))
        pers = ctx.enter_context(tc.tile_pool(name="pers", bufs=1))

        # ---- constants (wsb + first x quarter first: qkv b0 needs them) ----
        wsb = constp.tile([128, NCT, 384], BF16)
        nc.sync.dma_start(wsb[:], wT[:].rearrange("(a p) m -> p a m", p=128))
        xq = [constp.tile([128, NCT, 1024], BF16, name=f"xq{i}")
              for i in range(4)]

        def xs(ct, lo, n):
            """x slice [128, n] covering global token cols [lo, lo+n)."""
            q, l = lo // 1024, lo % 1024
            assert l + n <= 1024
            return xq[q][:, ct, l:l + n]

        nc.sync.dma_start(
            xq[0][:], xT[:, 0:1024].rearrange("(a p) m -> p a m", p=128))
        nc.sync.dma_start(
            xq[1][:], xT[:, 1024:2048].rearrange("(a p) m -> p a m", p=128))
        cos_sb = constp.tile([128, T], BF16)
        nc.sync.dma_start(cos_sb[:], cosT[:])
        sin_sb = constp.tile([128, T], BF16)
        nc.sync.dma_start(sin_sb[:], sinS[:])
        bqk_sb = constp.tile([128, 2], F32)
        nc.sync.dma_start(bqk_sb[:], bqk[:])
        wo_sb = constp.tile([128, C], BF16)
        nc.sync.dma_start(wo_sb[:], woT[:])
        nc.sync.dma_start(
            xq[2][:], xT[:, 2048:3072].rearrange("(a p) m -> p a m", p=128))
        nc.sync.dma_start(
            xq[3][:], xT[:, 3072:4096].rearrange("(a p) m -> p a m", p=128))
        # triangular causal mask for the diagonal 128-blocks, both heads:
        # [128, 2, 128], keep (1.0) iff col >= partition
        mtri = constp.tile([128, 2, 128], BF16)
        nc.vector.memset(mtri[:], 1.0)
        nc.gpsimd.affine_select(
            out=mtri[:], in_=mtri[:], compare_op=mybir.AluOpType.is_ge,
            fill=0.0, base=0, channel_multiplier=-1,
            pattern=[[0, 2], [1, 128]])

        # ---- persistent activations ----
        qT = pers.tile([128, TB], BF16)
        kT = pers.tile([128, TB], BF16)
        # V in [ktok, dim] layout; col 64 of each k-tile stays 1.0 so the
        # P@V matmul also produces sum(exp) in psO row 64.
        Vsb = [[pers.tile([128, NKT * (HD + 1)], BF16, name=f"Vsb{b}{h}")
                for h in range(2)] for b in range(B)]
        for b in range(B):
            for h in range(2):
                nc.vector.memset(Vsb[b][h][:], 1.0)
        OT = [pers.tile([128, T], BF16, name=f"OTb{b}") for b in range(B)]

        def qk_pair(psqkv, tca, tcb):
            """q and k projections for two 512-token chunks, shared
            stationaries."""
            tsa = slice(tca * QCH, (tca + 1) * QCH)
            tsb = slice(tcb * QCH, (tcb + 1) * QCH)
            for g in range(2):
                psa = psqkv.tile([128, QCH], F32, tag="ps",
                                 name=f"psq{tca}_{g}")
                psb = psqkv.tile([128, QCH], F32, tag="ps",
                                 name=f"psq{tcb}_{g}")
                for ct in range(NCT):
                    w = wsb[:, ct, g * 128:(g + 1) * 128]
                    nc.tensor.matmul(psa[:], w, xs(ct, tca * QCH, QCH),
                                     start=(ct == 0), stop=(ct == NCT - 1))
                    nc.tensor.matmul(psb[:], w, xs(ct, tcb * QCH, QCH),
                                     start=(ct == 0), stop=(ct == NCT - 1))
                dst = qT if g == 0 else kT
                if hasb:
                    nc.scalar.activation(dst[:, tsa], psa[:], AF.Identity,
                                         bias=bqk_sb[:, g:g + 1])
                    nc.scalar.activation(dst[:, tsb], psb[:], AF.Identity,
                                         bias=bqk_sb[:, g:g + 1])
                else:
                    nc.vector.tensor_copy(dst[:, tsa], psa[:])
                    nc.vector.tensor_copy(dst[:, tsb], psb[:])

        def v_tiles(psvp, b):
            """V for batch b directly in [ktok, dim] layout: stationary is
            the x tile, moving is the 128 Wv columns (both heads)."""
            for tt in range(NKT):
                lo = b * T + tt * 128
                psv = psvp.tile([128, 128], F32, tag="v", name=f"psv{b}{tt}")
                for ct in range(NCT):
                    nc.tensor.matmul(psv[:], xs(ct, lo, 128),
                                     wsb[:, ct, 256:384],
                                     start=(ct == 0), stop=(ct == NCT - 1))
                o = tt * (HD + 1)
                nc.vector.tensor_copy(Vsb[b][0][:, o:o + HD], psv[:, 0:HD])
                nc.vector.tensor_copy(Vsb[b][1][:, o:o + HD],
                                      psv[:, HD:2 * HD])

        def rope_b(swpp, rtp, zt, b, nm):
            bs = slice(b * T, (b + 1) * T)
            swp = swpp.tile([128, T], BF16, tag="swp", name=f"swp{nm}")
            for h in range(2):
                o = h * 64
                nc.sync.dma_start(swp[o:o + 32, :], zt[o + 32:o + 64, bs])
                nc.sync.dma_start(swp[o + 32:o + 64, :], zt[o:o + 32, bs])
            tmp = rtp.tile([128, T], BF16, tag="rt", name=f"rt{nm}")
            nc.vector.tensor_mul(tmp[:], swp[:], sin_sb[:])
            nc.vector.tensor_mul(zt[:, bs], zt[:, bs], cos_sb[:])
            nc.vector.tensor_add(zt[:, bs], zt[:, bs], tmp[:])

        def proj_qc(pss, ybn, b, qc):
            """Out-projection units for the 4 token-tiles of (b, qc)."""
            for tt in range(4 * qc, 4 * qc + 4):
                yb = ybn.tile([128, C], BF16, tag="yb", name=f"yb{b}{tt}")
                ps = pss.tile([128, 2 * QCH], F32, tag="s",
                              name=f"psy{b}{tt}")
                for ncol in range(2):
                    nc.tensor.matmul(
                        ps[:, ncol * QCH:(ncol + 1) * QCH],
                        OT[b][:, tt * 128:(tt + 1) * 128],
                        wo_sb[:, ncol * QCH:(ncol + 1) * QCH],
                        start=True, stop=True)
                if tt % 2 == 0:
                    nc.vector.tensor_copy(yb[:], ps[:])
                else:
                    nc.scalar.activation(yb[:], ps[:], AF.Copy)
                nc.sync.dma_start(
                    y[b * T + tt * 128:b * T + (tt + 1) * 128, :], yb[:])

        def attn_b(pools, b, ybn, pending):
            pss, pso, ptp, mbp, smol = pools
            PIPE = 3
            for qc in range(NQC):
                nk = 4 * (qc + 1) if mode == "causal" else NKT
                psO = [pso.tile([65, QCH], F32, tag=f"o{h}",
                                name=f"psO{h}_{b}{qc}") for h in range(2)]
                pts = {}

                def offs(kt):
                    if mode == "causal" and kt >= 4 * qc:
                        return (kt - 4 * qc) * 128
                    return 0

                def emit_pv(j, nk=nk, psO=psO, pts=pts):
                    st, sp = (j == 0), (j == nk - 1)
                    off = offs(j)
                    pt = pts.pop(j)
                    ptv = pt[:].rearrange("p (h q) -> p h q", q=QCH)
                    for h in range(2):
                        nc.tensor.matmul(
                            psO[h][:, off:QCH],
                            Vsb[b][h][:, j * (HD + 1):(j + 1) * (HD + 1)],
                            ptv[:, h, off:QCH], start=st, stop=sp)

                for kt in range(nk):
                    if kt == 1 and pending:
                        # flush the previous q-chunk's out-projection here:
                        # its OT inputs finished an entire kt-loop ago, so
                        # these matmuls never stall the PE queue
                        pending.pop(0)()
                    ks = slice(b * T + kt * 128, b * T + (kt + 1) * 128)
                    off = offs(kt)
                    qs = slice(b * T + qc * QCH + off, b * T + (qc + 1) * QCH)
                    psS = pss.tile([128, 2 * QCH], F32, tag="s",
                                   name=f"psS{b}{qc}{kt}")
                    # the two heads occupy disjoint PE row groups
                    # (partitions 0:64 / 64:128) -> emitted back-to-back
                    # they execute concurrently
                    nc.tensor.matmul(psS[:, off:QCH], kT[0:64, ks],
                                     qT[0:64, qs], start=True, stop=True)
                    nc.tensor.matmul(psS[:, QCH + off:2 * QCH],
                                     kT[64:128, ks], qT[64:128, qs],
                                     start=True, stop=True)
                    pt = ptp.tile([128, 2 * QCH], BF16, tag="pt",
                                  name=f"pt{b}{qc}{kt}")
                    psv_ = psS[:].rearrange("p (h q) -> p h q", q=QCH)
                    ptv = pt[:].rearrange("p (h q) -> p h q", q=QCH)
                    nc.scalar.activation(ptv[:, :, off:QCH],
                                         psv_[:, :, off:QCH], AF.Exp)
                    if mode == "causal" and kt >= 4 * qc:
                        nc.vector.tensor_mul(ptv[:, :, off:off + 128],
                                             ptv[:, :, off:off + 128],
                                             mtri[:])
                    elif mode == "bias":
                        mt = mbp.tile([128, QCH], BF16, tag="mb",
                                      name=f"mt{b}{qc}{kt}")
                        nc.sync.dma_start(
                            mt[:], maskT[kt * 128:(kt + 1) * 128,
                                         qc * QCH:(qc + 1) * QCH])
                        nc.vector.tensor_mul(ptv[:, 0, :], ptv[:, 0, :],
                                             mt[:])
                        nc.vector.tensor_mul(ptv[:, 1, :], ptv[:, 1, :],
                                             mt[:])
                    pts[kt] = pt
                    if kt >= PIPE:
                        emit_pv(kt - PIPE)
                for j in range(max(0, nk - PIPE), nk):
                    emit_pv(j)

                # normalize + evict; sum(exp) sits in row 64 of psO[h]
                oqs = slice(qc * QCH, (qc + 1) * QCH)
                for h in range(2):
                    nm = f"{b}{qc}{h}"
                    rw = smol.tile([65, QCH], F32, tag="rw", name=f"rw{nm}")
                    nc.vector.tensor_copy(rw[64:65, :], psO[h][64:65, :])
                    rz = smol.tile([1, QCH], F32, tag="rz", name=f"rz{nm}")
                    nc.sync.dma_start(rz[:], rw[64:65, :])
                    rr = smol.tile([1, QCH], F32, tag="rr", name=f"rr{nm}")
                    nc.vector.reciprocal_approx_fast(rr[:], rz[:])
                    bc = smol.tile([64, QCH], F32, tag="bc", name=f"bc{nm}")
                    nc.gpsimd.partition_broadcast(bc[:], rr[:])
                    if h == 0:
                        nc.vector.tensor_mul(OT[b][0:64, oqs],
                                             psO[h][0:64, :], bc[:])
                    else:
                        otmp = smol.tile([64, QCH], BF16, tag="otmp",
                                         name=f"otmp{nm}")
                        nc.vector.tensor_mul(otmp[:], psO[h][0:64, :], bc[:])
                        nc.sync.dma_start(OT[b][64:128, oqs], otmp[:])

                pending.append(
                    lambda b=b, qc=qc: proj_qc(pss, ybn, b, qc))

        # ---- phases ----
        with tc.tile_pool(name="psqkv", bufs=4, space="PSUM") as psqkv, \
             tc.tile_pool(name="psv", bufs=4, space="PSUM") as psvp, \
             tc.tile_pool(name="swp", bufs=2) as swpp, \
             tc.tile_pool(name="rtmp", bufs=2) as rtp:
            for b in range(B):
                qk_pair(psqkv, 4 * b + 0, 4 * b + 1)
                qk_pair(psqkv, 4 * b + 2, 4 * b + 3)
                v_tiles(psvp, b)
                rope_b(swpp, rtp, qT, b, f"q{b}")
                rope_b(swpp, rtp, kT, b, f"k{b}")

        with tc.tile_pool(name="pss", bufs=2, space="PSUM") as pss, \
             tc.tile_pool(name="pso", bufs=2, space="PSUM") as pso, \
             tc.tile_pool(name="ptp", bufs=4) as ptp, \
             tc.tile_pool(name="mbp", bufs=4) as mbp, \
             tc.tile_pool(name="smol", bufs=4) as smol, \
             tc.tile_pool(name="ybp", bufs=4) as ybp:
            pending = []
            for b in range(B):
                attn_b((pss, pso, ptp, mbp, smol), b, ybp, pending)
            for p in pending:
                p()


def _build_program(mode, hasb):
    key = (mode, hasb)
    if key in _PROG_CACHE:
        return _PROG_CACHE[key]
    nc = bacc.Bacc("TRN2", target_bir_lowering=False, debug=False,
                   num_devices=NCORES)
    dram = {
        "xT": nc.dram_tensor("xT", [C, TB], BF16, kind="ExternalInput").ap(),
        "wT": nc.dram_tensor("wT", [C, 384], BF16, kind="ExternalInput").ap(),
        "bqk": nc.dram_tensor("bqk", [128, 2], F32, kind="ExternalInput").ap(),
        "cosT": nc.dram_tensor("cosT", [128, T], BF16,
                               kind="ExternalInput").ap(),
        "sinS": nc.dram_tensor("sinS", [128, T], BF16,
                               kind="ExternalInput").ap(),
        "woT": nc.dram_tensor("woT", [128, C], BF16,
                              kind="ExternalInput").ap(),
        "y": nc.dram_tensor("y", [TB, C], BF16, kind="ExternalOutput").ap(),
    }
    if mode == "bias":
        dram["maskT"] = nc.dram_tensor("maskT", [T, T], BF16,
                                       kind="ExternalInput").ap()
    with tile.TileContext(nc) as tc:
        _emit(tc, mode, hasb, dram)
    nc.compile()
    _PROG_CACHE[key] = (nc, dram)
    return nc, dram


def _rope_tables():
    inv_freq = 1.0 / (10000.0 ** (np.arange(0, HD, 2, dtype=np.float64) / HD))
    freqs = np.arange(T, dtype=np.float64)[:, None] * inv_freq[None, :]
    cos = np.concatenate([np.cos(freqs), np.cos(freqs)], axis=-1)  # [T, 64]
    sin = np.concatenate([np.sin(freqs), np.sin(freqs)], axis=-1)
    cE = cos[:, 0::2].T  # [32, T] rows i -> dim 2i
    cO = cos[:, 1::2].T
    sE = sin[:, 0::2].T
    sO = sin[:, 1::2].T
    cosT = np.concatenate([cE, cO, cE, cO], axis=0)
    sinS = np.concatenate([-sE, sO, -sE, sO], axis=0)
    return (np.ascontiguousarray(cosT.astype(NPBF16)),
            np.ascontiguousarray(sinS.astype(NPBF16)))


def _detect_mode(mask):
    mb = mask.reshape(T, T)
    if np.array_equal(mb != 0, np.tril(np.ones((T, T), dtype=bool))):
        return "causal"
    if np.all(mb != 0):
        return "dense"
    return "bias"


def _prepare_in_maps(x, mask, Wqkv, bqkv, Wo, mode):
    xTn = np.ascontiguousarray(
        x.reshape(TB, C).T.astype(NPBF16))
    cosT, sinS = _rope_tables()
    scale = 1.0 / np.sqrt(np.float32(HD))

    evens = np.arange(0, HD, 2)
    odds = evens + 1

    in_maps = []
    for c in range(NCORES):
        h0, h1 = 2 * c, 2 * c + 1
        qrows = np.concatenate([h0 * HD + evens, h0 * HD + odds,
                                h1 * HD + evens, h1 * HD + odds])
        krows = C + qrows
        vrows = np.concatenate([2 * C + h0 * HD + np.arange(HD),
                                2 * C + h1 * HD + np.arange(HD)])
        wq = Wqkv[qrows, :] * scale
        wk = Wqkv[krows, :]
        wv = Wqkv[vrows, :]
        wT = np.ascontiguousarray(
            np.concatenate([wq, wk, wv], axis=0).T.astype(NPBF16))
        bqk = np.stack([bqkv[qrows] * scale, bqkv[krows]], axis=1)
        woT = np.ascontiguousarray(
            Wo[:, 128 * c:128 * (c + 1)].T.astype(NPBF16))
        im = {
            "xT": xTn, "wT": wT,
            "bqk": np.ascontiguousarray(bqk, dtype=np.float32),
            "cosT": cosT, "sinS": sinS, "woT": woT,
        }
        if mode == "bias":
            im["maskT"] = np.ascontiguousarray(
                (mask.reshape(T, T) != 0).T.astype(NPBF16))
        in_maps.append(im)
    return in_maps


def kernel(x, mask, Wqkv, bqkv, Wo, bo):
    x = np.asarray(x, dtype=np.float32)
    mask = np.asarray(mask)
    Wqkv = np.asarray(Wqkv, dtype=np.float32)
    bqkv = np.asarray(bqkv, dtype=np.float32)
    Wo = np.asarray(Wo, dtype=np.float32)
    bo = np.asarray(bo, dtype=np.float32)

    mode = _detect_mode(mask)
    hasb = bool(np.any(bqkv[:2 * C] != 0.0))
    nc, dram = _build_program(mode, hasb)
    in_maps = _prepare_in_maps(x, mask, Wqkv, bqkv, Wo, mode)

    res = run_bass_kernel_spmd(nc, in_maps, core_ids=list(range(NCORES)))
    y = np.zeros((TB, C), dtype=np.float32)
    for c in range(NCORES):
        y += res.results[c]["y"].astype(np.float32)
    bv = bqkv[2 * C:3 * C]
    y += (bo + bv @ Wo.T)[None, :]
    return y.reshape(B, T, C)
